# revision 2
# baseline (speedup 1.0000x reference)
"""Trainium2 Bass kernel for nn_Attention_31396210933853 (v3).

Computation (B=32, S=4096, D=512):
    eij[b,s] = sum_d x[b,s,d]*kernel[d] + bias[s]
    a        = exp(tanh(eij)) * mask
    out[b,d] = sum_s a[b,s]*x[b,s,d] / (sum_s a[b,s] + EPS)

v3 strategy vs v1 (114.8us):
  * x converted to 16-bit on host -> HBM read halves: 16 MiB/core,
    DMA floor ~47us at ~358 GB/s per core.
  * s-on-partition layout, tiles (128, J*512); J=4 -> 512KB DMAs
    (32 dispatches/core instead of 64+, ~650ns each on sync queue).
  * Pass A (eij raw) bulk split across engines per 512-wide unit:
      R1: DVE scalar_tensor_tensor (mult+accum, ~700ns/unit)
      R2: Pool tensor_mul -> fp32 scratch, ACT Copy+accum_out reduce
          (~1200ns Pool + ~990ns ACT per unit)
    so no single vector engine exceeds the DMA floor.
  * Activation chain batched per sample on (128, 32): bias add (DVE),
    tanh+exp (ACT), mask mult (DVE) -> tiny vs v1's per-tile ops.
  * Pass B on PE: per unit one matmul (a col stationary, x moving,
    N=512, ~216ns) accumulating U[b] in PSUM; den = one ones-matmul
    per sample over a_all (128,32). Normalization deferred to epilog.

Sharding: data-parallel over batch, 4 samples per core on 8 cores.
"""
import numpy as np

import concourse.bass as bass
import concourse.bacc as bacc
import concourse.tile as tile
from concourse import mybir
from concourse.bass_utils import run_bass_kernel_spmd

B, S, D = 32, 4096, 512
N_CORES = 8
BC = B // N_CORES        # samples per core
P = 128                  # SBUF partitions
J = 4                    # 512-wide units per partition row per tile
T = S // (P * J)         # x tiles per sample (8)
UNITS = T * J            # 512-wide units per sample (32)
XBUFS = 20               # x-tile pipeline depth (512KB each)
EPS = 1e-7

# dtype for x / a / kernel on device: "f16" or "bf16"
X_DTYPE = "f16"
# Units (by global index mod PERIOD) routed to Pool+ACT instead of DVE.
R2_PERIOD = 8
R2_SLOTS = (1, 4, 6)

TRACE = False
LAST_RESULTS = None

_PROGRAM_CACHE = {}


def _build_program(xdt_name, r2_period, r2_slots):
    f32 = mybir.dt.float32
    xdt = mybir.dt.float16 if xdt_name == "f16" else mybir.dt.bfloat16
    FT = mybir.ActivationFunctionType
    OP = mybir.AluOpType

    nc = bacc.Bacc(
        "TRN2", target_bir_lowering=False, debug=False, num_devices=N_CORES
    )
    x_d = nc.dram_tensor("x", [BC, T, P, J * D], xdt, kind="ExternalInput")
    kb_d = nc.dram_tensor("kb", [1, D], xdt, kind="ExternalInput")
    bias_d = nc.dram_tensor("bias_t", [P, UNITS], f32, kind="ExternalInput")
    mask_d = nc.dram_tensor("mask_t", [BC, P, UNITS], f32, kind="ExternalInput")
    ones_d = nc.dram_tensor("ones", [P, 1], xdt, kind="ExternalInput")
    out_d = nc.dram_tensor("out", [1, BC * D], f32, kind="ExternalOutput")

    with tile.TileContext(nc) as tc:
        with (
            tc.tile_pool(name="xp", bufs=XBUFS) as xp,
            tc.tile_pool(name="cons", bufs=1) as cons,
            tc.tile_pool(name="wd", bufs=4) as wdp,
            tc.tile_pool(name="wg", bufs=4) as wgp,
            tc.tile_pool(name="small", bufs=8) as small,
            tc.tile_pool(name="fin", bufs=4) as fin,
            tc.tile_pool(name="psum", bufs=1, space="PSUM") as psp,
        ):
            kb = cons.tile([P, D], xdt)
            nc.gpsimd.dma_start(out=kb, in_=kb_d.ap().to_broadcast([P, D]))
            bias_t = cons.tile([P, UNITS], f32)
            nc.gpsimd.dma_start(out=bias_t, in_=bias_d[:])
            mask_all = cons.tile([P, BC * UNITS], f32)
            for b in range(BC):
                nc.gpsimd.dma_start(
                    out=mask_all[:, b * UNITS : (b + 1) * UNITS],
                    in_=mask_d[b],
                )
            ones = cons.tile([P, 1], xdt)
            nc.gpsimd.dma_start(out=ones, in_=ones_d[:])
            out_row = cons.tile([1, BC * D], f32)

            u_ps = [
                psp.tile([1, D], f32, name=f"u_ps{b}", tag=f"u{b}")
                for b in range(BC)
            ]
            den_ps = psp.tile([1, BC * UNITS], f32, tag="den")

            unit_ctr = 0
            for b in range(BC):
                eraw = small.tile([P, UNITS], f32, name=f"eraw{b}")
                x_tiles = []
                for t in range(T):
                    x_t = xp.tile([P, J * D], xdt)
                    nc.sync.dma_start(out=x_t, in_=x_d[b, t])
                    x_tiles.append(x_t)
                    for j in range(J):
                        col = t * J + j
                        xs = x_t[:, j * D : (j + 1) * D]
                        if (unit_ctr % r2_period) in r2_slots:
                            wg = wgp.tile([P, D], f32)
                            nc.gpsimd.tensor_mul(wg, xs, kb)
                            nc.scalar.activation(
                                wg, wg, FT.Copy,
                                accum_out=eraw[:, col : col + 1],
                            )
                        else:
                            wd = wdp.tile([P, D], xdt)
                            nc.vector.scalar_tensor_tensor(
                                out=wd,
                                in0=xs,
                                scalar=0.0,
                                in1=kb,
                                op0=OP.bypass,
                                op1=OP.mult,
                                accum_out=eraw[:, col : col + 1],
                            )
                        unit_ctr += 1

                eij = small.tile([P, UNITS], f32, name=f"eij{b}")
                nc.vector.tensor_add(eij, eraw, bias_t)
                th = small.tile([P, UNITS], f32, name=f"th{b}")
                nc.scalar.activation(th, eij, FT.Tanh)
                ex = small.tile([P, UNITS], f32, name=f"ex{b}")
                nc.scalar.activation(ex, th, FT.Exp)
                a_all = small.tile([P, UNITS], xdt, name=f"a{b}")
                nc.vector.tensor_mul(
                    a_all, ex, mask_all[:, b * UNITS : (b + 1) * UNITS]
                )

                nc.tensor.matmul(
                    den_ps[:, b * UNITS : (b + 1) * UNITS],
                    lhsT=ones,
                    rhs=a_all,
                    start=True,
                    stop=True,
                )
                for t in range(T):
                    for j in range(J):
                        col = t * J + j
                        nc.tensor.matmul(
                            u_ps[b][:, :],
                            lhsT=a_all[:, col : col + 1],
                            rhs=x_tiles[t][:, j * D : (j + 1) * D],
                            start=(col == 0),
                            stop=(col == UNITS - 1),
                        )

                denr = fin.tile([1, 1], f32, tag="denr", name=f"denr{b}")
                nc.vector.tensor_reduce(
                    out=denr,
                    in_=den_ps[:, b * UNITS : (b + 1) * UNITS],
                    axis=mybir.AxisListType.X,
                    op=OP.add,
                )
                deno = fin.tile([1, 1], f32, tag="deno", name=f"deno{b}")
                nc.vector.tensor_scalar_add(deno, denr, EPS)
                rec = fin.tile([1, 1], f32, tag="rec", name=f"rec{b}")
                nc.vector.reciprocal(rec, deno)
                nc.vector.tensor_scalar_mul(
                    out_row[:, b * D : (b + 1) * D], u_ps[b], rec
                )

            nc.sync.dma_start(out=out_d[:], in_=out_row)

    nc.compile()
    return nc


def _get_program():
    key = (X_DTYPE, R2_PERIOD, R2_SLOTS)
    if key not in _PROGRAM_CACHE:
        _PROGRAM_CACHE[key] = _build_program(*key)
    return _PROGRAM_CACHE[key]


def _np_dtype():
    if X_DTYPE == "f16":
        return np.float16
    import ml_dtypes

    return ml_dtypes.bfloat16


def _prep_inputs(x, kern, bias, mask):
    """Host-side sharding/layout marshaling."""
    ndt = _np_dtype()
    x16 = np.asarray(x, dtype=np.float32).astype(ndt)
    kb = np.ascontiguousarray(
        np.asarray(kern, dtype=np.float32).astype(ndt)[None, :]
    )
    bias_t = np.ascontiguousarray(
        np.asarray(bias, dtype=np.float32)
        .reshape(T, P, J)
        .transpose(1, 0, 2)
        .reshape(P, UNITS)
    )
    mask_f = np.asarray(mask).astype(np.float32)
    ones = np.ones((P, 1), dtype=ndt)
    in_maps = []
    for i in range(N_CORES):
        xs = x16[i * BC : (i + 1) * BC].reshape(BC, T, P, J * D)
        ms = (
            mask_f[i * BC : (i + 1) * BC]
            .reshape(BC, T, P, J)
            .transpose(0, 2, 1, 3)
            .reshape(BC, P, UNITS)
        )
        in_maps.append(
            {
                "x": xs,
                "kb": kb,
                "bias_t": bias_t,
                "mask_t": np.ascontiguousarray(ms),
                "ones": ones,
            }
        )
    return in_maps


def kernel(x, kernel, bias, mask):
    global LAST_RESULTS
    nc = _get_program()
    in_maps = _prep_inputs(x, kernel, bias, mask)
    res = run_bass_kernel_spmd(nc, in_maps, list(range(N_CORES)), trace=TRACE)
    LAST_RESULTS = res
    out = np.concatenate(
        [res.results[i]["out"].reshape(BC, D) for i in range(N_CORES)], axis=0
    )
    return out.astype(np.float32, copy=False)


# revision 8
# speedup vs baseline: 1.0140x; 1.0140x over previous
"""Trainium2 Bass kernel for nn_Attention_31396210933853 (v3).

Computation (B=32, S=4096, D=512):
    eij[b,s] = sum_d x[b,s,d]*kernel[d] + bias[s]
    a        = exp(tanh(eij)) * mask
    out[b,d] = sum_s a[b,s]*x[b,s,d] / (sum_s a[b,s] + EPS)

v3 strategy vs v1 (114.8us):
  * x converted to 16-bit on host -> HBM read halves: 16 MiB/core,
    DMA floor ~47us at ~358 GB/s per core.
  * s-on-partition layout, tiles (128, J*512); J=4 -> 512KB DMAs
    (32 dispatches/core instead of 64+, ~650ns each on sync queue).
  * Pass A (eij raw) bulk split across engines per 512-wide unit:
      R1: DVE scalar_tensor_tensor (mult+accum, ~700ns/unit)
      R2: Pool tensor_mul -> fp32 scratch, ACT Copy+accum_out reduce
          (~1200ns Pool + ~990ns ACT per unit)
    so no single vector engine exceeds the DMA floor.
  * Activation chain batched per sample on (128, 32): bias add (DVE),
    tanh+exp (ACT), mask mult (DVE) -> tiny vs v1's per-tile ops.
  * Pass B on PE: per unit one matmul (a col stationary, x moving,
    N=512, ~216ns) accumulating U[b] in PSUM; den = one ones-matmul
    per sample over a_all (128,32). Normalization deferred to epilog.

Sharding: data-parallel over batch, 4 samples per core on 8 cores.
"""
import numpy as np

import concourse.bass as bass
import concourse.bacc as bacc
import concourse.tile as tile
from concourse import mybir
from concourse.bass_utils import run_bass_kernel_spmd

B, S, D = 32, 4096, 512
N_CORES = 8
BC = B // N_CORES        # samples per core
P = 128                  # SBUF partitions
J = 4                    # 512-wide units per partition row per tile
T = S // (P * J)         # x tiles per sample (8)
UNITS = T * J            # 512-wide units per sample (32)
XBUFS = 20               # x-tile pipeline depth (512KB each)
EPS = 1e-7

# dtype for x / a / kernel on device: "f16" or "bf16"
X_DTYPE = "f16"
# Units (by global index mod PERIOD) routed to Pool+ACT instead of DVE.
R2_PERIOD = 8
R2_SLOTS = (1, 4, 6)

TRACE = False
LAST_RESULTS = None

_PROGRAM_CACHE = {}


def _build_program(xdt_name, r2_period, r2_slots):
    f32 = mybir.dt.float32
    xdt = mybir.dt.float16 if xdt_name == "f16" else mybir.dt.bfloat16
    FT = mybir.ActivationFunctionType
    OP = mybir.AluOpType

    nc = bacc.Bacc(
        "TRN2", target_bir_lowering=False, debug=False, num_devices=N_CORES
    )
    x_d = nc.dram_tensor("x", [BC, T, P, J * D], xdt, kind="ExternalInput")
    kb_d = nc.dram_tensor("kb", [1, D], xdt, kind="ExternalInput")
    bias_d = nc.dram_tensor("bias_t", [P, UNITS], f32, kind="ExternalInput")
    mask_d = nc.dram_tensor("mask_t", [BC, P, UNITS], f32, kind="ExternalInput")
    ones_d = nc.dram_tensor("ones", [P, 1], xdt, kind="ExternalInput")
    out_d = nc.dram_tensor("out", [1, BC * D], f32, kind="ExternalOutput")

    with tile.TileContext(nc) as tc:
        with (
            tc.tile_pool(name="xp", bufs=XBUFS) as xp,
            tc.tile_pool(name="cons", bufs=1) as cons,
            tc.tile_pool(name="wd", bufs=2, space="PSUM") as wdp,
            tc.tile_pool(name="wg", bufs=3) as wgp,
            tc.tile_pool(name="wa", bufs=2, space="PSUM") as wap,
            tc.tile_pool(name="small", bufs=8) as small,
            tc.tile_pool(name="fin", bufs=4) as fin,
            tc.tile_pool(name="psum", bufs=2, space="PSUM") as psp,
            tc.tile_pool(name="dpsum", bufs=2, space="PSUM") as dpsp,
        ):
            kb = cons.tile([P, D], xdt)
            nc.gpsimd.dma_start(out=kb, in_=kb_d.ap().to_broadcast([P, D]))
            bias_t = cons.tile([P, UNITS], f32)
            nc.gpsimd.dma_start(out=bias_t, in_=bias_d[:])
            mask_all = cons.tile([P, BC * UNITS], f32)
            for b in range(BC):
                nc.gpsimd.dma_start(
                    out=mask_all[:, b * UNITS : (b + 1) * UNITS],
                    in_=mask_d[b],
                )
            ones = cons.tile([P, 1], xdt)
            nc.gpsimd.dma_start(out=ones, in_=ones_d[:])
            out_row = cons.tile([1, BC * D], f32)

            unit_ctr = 0
            for b in range(BC):
                u_ps = psp.tile([1, D], f32, name="u_ps")
                den_ps = dpsp.tile([1, UNITS], f32, name="den_ps")
                eraw = small.tile([P, UNITS], f32, name="eraw")
                x_tiles = []
                for t in range(T):
                    x_t = xp.tile([P, J * D], xdt)
                    nc.sync.dma_start(out=x_t, in_=x_d[b, t])
                    x_tiles.append(x_t)
                    for j in range(J):
                        col = t * J + j
                        xs = x_t[:, j * D : (j + 1) * D]
                        if (unit_ctr % r2_period) in r2_slots:
                            wg = wgp.tile([P, D], xdt)
                            nc.gpsimd.tensor_mul(wg, xs, kb)
                            wa = wap.tile([P, D], f32)
                            nc.scalar.activation(
                                wa, wg, FT.Copy,
                                accum_out=eraw[:, col : col + 1],
                            )
                        else:
                            wd = wdp.tile([P, D], f32)
                            nc.vector.scalar_tensor_tensor(
                                out=wd,
                                in0=xs,
                                scalar=0.0,
                                in1=kb,
                                op0=OP.bypass,
                                op1=OP.mult,
                                accum_out=eraw[:, col : col + 1],
                            )
                        unit_ctr += 1

                eij = small.tile([P, UNITS], f32, name="eij")
                nc.vector.tensor_add(eij, eraw, bias_t)
                th = small.tile([P, UNITS], f32, name="th")
                nc.scalar.activation(th, eij, FT.Tanh)
                ex = small.tile([P, UNITS], f32, name="ex")
                nc.scalar.activation(ex, th, FT.Exp)
                a_all = small.tile([P, UNITS], xdt, name="a_all")
                nc.vector.tensor_mul(
                    a_all, ex, mask_all[:, b * UNITS : (b + 1) * UNITS]
                )

                nc.tensor.matmul(
                    den_ps[:, :],
                    lhsT=ones,
                    rhs=a_all,
                    start=True,
                    stop=True,
                )
                for t in range(T):
                    for j in range(J):
                        col = t * J + j
                        nc.tensor.matmul(
                            u_ps[:, :],
                            lhsT=a_all[:, col : col + 1],
                            rhs=x_tiles[t][:, j * D : (j + 1) * D],
                            start=(col == 0),
                            stop=(col == UNITS - 1),
                        )

                denr = fin.tile([1, 1], f32, name="denr")
                nc.vector.tensor_reduce(
                    out=denr,
                    in_=den_ps[:, :],
                    axis=mybir.AxisListType.X,
                    op=OP.add,
                )
                deno = fin.tile([1, 1], f32, name="deno")
                nc.vector.tensor_scalar_add(deno, denr, EPS)
                rec = fin.tile([1, 1], f32, name="rec")
                nc.vector.reciprocal(rec, deno)
                nc.vector.tensor_scalar_mul(
                    out_row[:, b * D : (b + 1) * D], u_ps, rec
                )

            nc.sync.dma_start(out=out_d[:], in_=out_row)

    nc.compile()
    return nc


def _get_program():
    key = (X_DTYPE, R2_PERIOD, R2_SLOTS)
    if key not in _PROGRAM_CACHE:
        _PROGRAM_CACHE[key] = _build_program(*key)
    return _PROGRAM_CACHE[key]


def _np_dtype():
    if X_DTYPE == "f16":
        return np.float16
    import ml_dtypes

    return ml_dtypes.bfloat16


def _prep_inputs(x, kern, bias, mask):
    """Host-side sharding/layout marshaling."""
    ndt = _np_dtype()
    x16 = np.asarray(x, dtype=np.float32).astype(ndt)
    kb = np.ascontiguousarray(
        np.asarray(kern, dtype=np.float32).astype(ndt)[None, :]
    )
    bias_t = np.ascontiguousarray(
        np.asarray(bias, dtype=np.float32)
        .reshape(T, P, J)
        .transpose(1, 0, 2)
        .reshape(P, UNITS)
    )
    mask_f = np.asarray(mask).astype(np.float32)
    ones = np.ones((P, 1), dtype=ndt)
    in_maps = []
    for i in range(N_CORES):
        xs = x16[i * BC : (i + 1) * BC].reshape(BC, T, P, J * D)
        ms = (
            mask_f[i * BC : (i + 1) * BC]
            .reshape(BC, T, P, J)
            .transpose(0, 2, 1, 3)
            .reshape(BC, P, UNITS)
        )
        in_maps.append(
            {
                "x": xs,
                "kb": kb,
                "bias_t": bias_t,
                "mask_t": np.ascontiguousarray(ms),
                "ones": ones,
            }
        )
    return in_maps


def kernel(x, kernel, bias, mask):
    global LAST_RESULTS
    nc = _get_program()
    in_maps = _prep_inputs(x, kernel, bias, mask)
    res = run_bass_kernel_spmd(nc, in_maps, list(range(N_CORES)), trace=TRACE)
    LAST_RESULTS = res
    out = np.concatenate(
        [res.results[i]["out"].reshape(BC, D) for i in range(N_CORES)], axis=0
    )
    return out.astype(np.float32, copy=False)
